# revision 1
# baseline (speedup 1.0000x reference)
"""Trainium2 Bass kernel v2 for nn_DecoderBlock (B=8, S=1024, D=256, H=4 heads
of full width 256, FF=1024). Data-parallel: 1 batch element per core.

Key ideas over v1:
- kt-trick: scores = x^T (wq^T wk) x, with M_h = wq_h^T wk_h folded on the
  host. One projection kt = M x per head instead of q AND k, and the scores
  matmul consumes the fp8 input xp directly as the moving operand.
- fp8 DoubleRow matmuls (2 MACs/cycle) for kt, V, scores, Z, att@V, wo and
  FFN2. Power-of-2 scales keep fp8 in range: weights x64, exp outputs x16
  (via a ln16 activation bias); the residual streams stay bf16 and LayerNorm
  is scale-invariant so the wo/ff2 PSUM scale (4096) is divided out in the
  single fused stt evacuation.
- All ACT functions stay inside the natural_log_exp_and_others table set
  (rstd = exp(-0.5*ln(var+eps))) -> zero mid-kernel ACT_TABLE_LOADs.
- Element-wise work spread across DVE / ACT / GpSimd (masks, squares and half
  the LN applies run on GpSimd; it cannot touch PSUM so PSUM-reading ops stay
  on DVE/ACT).
- PE warm dummies bridge the LN1 chain so the HAM clock-gate stays 8/8.
"""

import numpy as np
import ml_dtypes

import concourse.bass as bass
import concourse.mybir as mybir
import concourse.tile as tile
from concourse import bacc
from concourse.bass_utils import run_bass_kernel_spmd

F32 = mybir.dt.float32
BF16 = mybir.dt.bfloat16
FP8 = mybir.dt.float8e4
AF = mybir.ActivationFunctionType
ALU = mybir.AluOpType
DR = mybir.MatmulPerfMode.DoubleRow

N_CORES = 8
B, S, D, H, E, HE, FF = 8, 1024, 256, 4, 256, 1024, 1024
SC = 512
NJ = S // SC          # 2
ND = D // 128         # 2
NF = FF // 128        # 8
LN_EPS = 1e-5
LN16 = float(np.log(16.0))
FP8MAX = 240.0

_CACHE = {}


def _build():
    nc = bacc.Bacc("TRN2", target_bir_lowering=False, debug=False,
                   num_devices=N_CORES)

    # ---- DRAM parameters (packed to minimize DMA descriptor count) ----
    xp_d = nc.dram_tensor("xp", [128, 2 * S], FP8, kind="ExternalInput")
    xb_d = nc.dram_tensor("xb", [128, 2 * S], BF16, kind="ExternalInput")
    # wpk: mq0|wv0|mq1|wv1|mq2|wv2|mq3|wv3|wo0|wo1|wo2|wo3 (512 cols each)
    wpk_d = nc.dram_tensor("wpk", [128, 12 * 512], FP8, kind="ExternalInput")
    f1_d = nc.dram_tensor("f1", [128, 2 * FF], BF16, kind="ExternalInput")
    f2_d = nc.dram_tensor("f2", [128, 4 * 2 * D], FP8, kind="ExternalInput")
    cen_d = nc.dram_tensor("cen", [128, 256], BF16, kind="ExternalInput")
    cs_d = nc.dram_tensor("cs", [128, 16], F32, kind="ExternalInput")
    out_d = nc.dram_tensor("out", [ND, 128, S], F32, kind="ExternalOutput")

    def v3(ap, two, size):
        return ap.rearrange("p (two s) -> p two s", two=two)

    with tile.TileContext(nc) as tc:
        with tc.tile_pool(name="consts", bufs=1) as consts, \
             tc.tile_pool(name="acts", bufs=1) as acts, \
             tc.tile_pool(name="work", bufs=2) as work, \
             tc.tile_pool(name="eps_", bufs=3) as epool, \
             tc.tile_pool(name="outp", bufs=2) as outp, \
             tc.tile_pool(name="psP", bufs=2, space="PSUM") as psP, \
             tc.tile_pool(name="psO", bufs=3, space="PSUM") as psO, \
             tc.tile_pool(name="psZ", bufs=1, space="PSUM") as psZ:

            # ---- weights / consts into SBUF (packed, few descriptors) ----
            wpkt = consts.tile([128, 12 * 512], FP8, tag="wpk", name="wpk")
            xpt = consts.tile([128, 2 * S], FP8, tag="xp", name="xp")
            xbpk = consts.tile([128, 2 * S], BF16, tag="xb", name="xb")
            f1pk = consts.tile([128, 2 * FF], BF16, tag="f1", name="f1")
            f2pk = consts.tile([128, 8 * D], FP8, tag="f2", name="f2")
            cent = consts.tile([128, 256], BF16, tag="cen", name="cen")
            cst = consts.tile([128, 16], F32, tag="cs", name="cs")

            mq = [wpkt[:, (2 * h) * 512:(2 * h) * 512 + 512]
                  for h in range(H)]
            wv = [wpkt[:, (2 * h + 1) * 512:(2 * h + 1) * 512 + 512]
                  for h in range(H)]
            wo = [wpkt[:, (8 + h) * 512:(8 + h) * 512 + 512]
                  for h in range(H)]
            xbt = [xbpk[:, d0 * S:(d0 + 1) * S] for d0 in range(ND)]
            f1 = [f1pk[:, d0 * FF:(d0 + 1) * FF] for d0 in range(ND)]
            f2 = [f2pk[:, c * 2 * D:(c + 1) * 2 * D] for c in range(NF // 2)]

            nc.sync.dma_start(out=wpkt[:, 0:1024], in_=wpk_d[:, 0:1024])
            nc.sync.dma_start(out=xpt[:], in_=xp_d[:])
            nc.sync.dma_start(out=wpkt[:, 1024:4096], in_=wpk_d[:, 1024:4096])
            nc.sync.dma_start(out=xbpk[:], in_=xb_d[:])
            nc.sync.dma_start(out=wpkt[:, 4096:6144], in_=wpk_d[:, 4096:6144])
            nc.sync.dma_start(out=cst[:], in_=cs_d[:])
            nc.sync.dma_start(out=cent[:], in_=cen_d[:])
            nc.sync.dma_start(out=f1pk[:], in_=f1_d[:])
            nc.sync.dma_start(out=f2pk[:], in_=f2_d[:])

            # per-partition scalar views of the packed consts
            ln1_g = [cst[:, 0 + d0:1 + d0] for d0 in range(ND)]
            ln1_b = [cst[:, 2 + d0:3 + d0] for d0 in range(ND)]
            ln2_g = [cst[:, 4 + d0:5 + d0] for d0 in range(ND)]
            ln2_b = [cst[:, 6 + d0:7 + d0] for d0 in range(ND)]
            f1b = [cst[:, 8 + f0:9 + f0] for f0 in range(NF)]

            # ---- small consts ----
            ones_dr = consts.tile([128, 256], FP8, tag="ones", name="ones")
            nc.vector.memset(ones_dr[:], 1.0)
            invd = consts.tile([128, 128], BF16, tag="invd", name="invd")
            nc.vector.memset(invd[:], 1.0 / D)
            onesw = consts.tile([128, 128], BF16, tag="onesw", name="onesw")
            nc.vector.memset(onesw[:], 1.0)
            eps_t = consts.tile([128, 1], F32, tag="eps", name="eps")
            nc.vector.memset(eps_t[:], LN_EPS)
            ln16_t = consts.tile([128, 1], F32, tag="ln16", name="ln16")
            nc.vector.memset(ln16_t[:], LN16)
            zero_t = consts.tile([128, 1], F32, tag="zero", name="zero")
            nc.vector.memset(zero_t[:], 0.0)

            # causal masks (bf16; exact 0/1)
            zcf = consts.tile([128, 256], F32, tag="zcf", name="zcf")
            nc.gpsimd.memset(zcf[:, 0:128], 0.0)
            nc.gpsimd.memset(zcf[:, 128:256], 1.0)
            nc.gpsimd.affine_select(
                out=zcf[:, 128:256], in_=zcf[:, 128:256],
                compare_op=ALU.is_ge, fill=0.0,
                base=0, pattern=[[1, 128]], channel_multiplier=-1)
            zcmask = consts.tile([128, 256], BF16, tag="zcm", name="zcm")
            nc.vector.tensor_copy(out=zcmask[:], in_=zcf[:])
            cmask = consts.tile([128, 128], BF16, tag="cm", name="cm")
            nc.vector.tensor_copy(out=cmask[:], in_=zcf[:, 128:256])

            # Dummy exp: makes the exp_and_others table the FIRST set the
            # ACT table pass sees (otherwise the v-quad Copy picks another
            # set and the first real exp pays a mid-attention reload).
            expscr = consts.tile([128, 1], F32, tag="expscr", name="expscr")
            nc.scalar.activation(out=expscr[:], in_=eps_t[:], func=AF.Exp)

            def warm_pe(n, width=128):
                for _ in range(n):
                    wp = psO.tile([128, width], F32, tag="o", name="warm")
                    nc.tensor.matmul(wp[:], onesw[:], onesw[:, 0:width],
                                     start=True, stop=True)

            warm_pe(14)

            # ================= attention =================
            ktp = [acts.tile([128, 2 * S], FP8, tag=f"ktp{i}", name=f"ktp{i}")
                   for i in range(2)]
            vq = [[acts.tile([128, 4 * E], FP8, tag=f"vq{i}{q}",
                             name=f"vq{i}{q}") for q in range(2)]
                  for i in range(2)]
            ontp = [acts.tile([128, 2 * S], FP8, tag=f"ont{h}",
                              name=f"ont{h}") for h in range(H)]

            def kt_thunk(h, j):
                """kt[:, jcols] = (M_h @ x): DR matmuls into a 2-bank pair,
                one DVE evac (x 1/16) into ktp."""
                i = h % 2
                cols = slice(j * SC, (j + 1) * SC)
                pair = psP.tile([128, 1024], F32, tag="pair", name=f"kt{h}{j}")
                mv = mq[h].rearrange("p (two d) -> p two d", two=2)
                xv = xpt[:].rearrange("p (two s) -> p two s", two=2)
                for d0 in range(ND):
                    nc.tensor.matmul(
                        pair[:, d0 * SC:d0 * SC + SC],
                        mv[:, :, d0 * 128:(d0 + 1) * 128],
                        xv[:, :, cols], start=True, stop=True, perf_mode=DR)
                kv = ktp[i][:].rearrange("p (two s) -> p two s", two=2)
                pv = pair[:].rearrange("p (two s) -> p two s", two=2)
                nc.vector.tensor_scalar(
                    out=kv[:, :, cols], in0=pv, scalar1=1.0 / 16.0,
                    scalar2=None, op0=ALU.mult)

            def v_thunk(h, q):
                """v quad q (t0 = 4q..4q+3): 4 DR matmuls into a 2-bank pair
                (two halves per bank), one ACT copy into vq."""
                i = h % 2
                pair = psP.tile([128, 1024], F32, tag="pair", name=f"v{h}{q}")
                xv = xpt[:].rearrange("p (two s) -> p two s", two=2)
                wvv = wv[h].rearrange("p (two e) -> p two e", two=2)
                for t in range(4):
                    t0 = 4 * q + t
                    nc.tensor.matmul(
                        pair[:, t * E:(t + 1) * E],
                        xv[:, :, t0 * 128:(t0 + 1) * 128],
                        wvv, start=(t % 2 == 0), stop=(t % 2 == 1),
                        perf_mode=DR, skip_group_check=True)
                nc.scalar.activation(out=vq[i][q][:], in_=pair[:],
                                     func=AF.Copy)

            def head_thunks(h):
                return [lambda: kt_thunk(h, 0), lambda: v_thunk(h, 0),
                        lambda: kt_thunk(h, 1), lambda: v_thunk(h, 1)]

            for t in head_thunks(0):
                t()

            xv = xpt[:].rearrange("p (two s) -> p two s", two=2)
            ov = ones_dr[:].rearrange("p (two s) -> p two s", two=2)

            class JCtx:
                """Z/O accumulation context for one (head, chunk)."""

                def __init__(self, h, j):
                    self.h, self.j = h, j
                    self.cmax = 2 * (j + 1)
                    self.zp = None
                    self.op = None

            def emit_zo(item):
                ctx, c, ep, off, w = item
                if c == 0:
                    ctx.zp = psZ.tile([128, SC], F32, tag="z",
                                      name=f"z{ctx.h}{ctx.j}")
                    ctx.op = [psO.tile([128, SC], F32, tag="o",
                                       name=f"o{ctx.h}{ctx.j}{e}")
                              for e in range(2)]
                last = (c == ctx.cmax - 1)
                ev = ep[:].rearrange("p (two s) -> p two s", two=2)
                er = ev[:, :, off:off + w]
                nc.tensor.matmul(
                    ctx.zp[:, off:off + w], ov, er,
                    start=(c == 0), stop=last,
                    perf_mode=DR, skip_group_check=True)
                qv = vq[ctx.h % 2][c // 2][:].rearrange(
                    "p (four e) -> p four e", four=4)
                for e0 in range(2):
                    nc.tensor.matmul(
                        ctx.op[e0][:, off:off + w],
                        qv[:, (c % 2) * 2:(c % 2) * 2 + 2,
                           e0 * 128:(e0 + 1) * 128],
                        er, start=(c == 0), stop=last,
                        perf_mode=DR, skip_group_check=True)
                if last:
                    # normalize: ontp = op * (1/zp)
                    zb = work.tile([128, SC], F32, tag="zb", name="zb")
                    nc.vector.reciprocal_approx_fast(out=zb[:],
                                                     in_=ctx.zp[:])
                    onv = ontp[ctx.h][:].rearrange("p (two s) -> p two s",
                                                   two=2)
                    cols = slice(ctx.j * SC, (ctx.j + 1) * SC)
                    for e0 in range(2):
                        nc.vector.tensor_mul(
                            out=onv[:, e0, cols], in0=ctx.op[e0][:],
                            in1=zb[:])

            pend = []
            for h in range(H):
                i = h % 2
                nxt = head_thunks(h + 1) if h + 1 < H else []
                ndone = 0
                it = 0
                kv = ktp[i][:].rearrange("p (two s) -> p two s", two=2)
                for j in range(NJ):
                    ctx = JCtx(h, j)
                    for c in range(ctx.cmax):
                        start_col = max(SC * j, 256 * c)
                        off = start_col - SC * j
                        w = SC - off
                        sp = psP.tile([128, 1024], F32, tag="pair",
                                      name=f"s{h}{j}{c}")
                        sv = sp[:].rearrange("p (two s) -> p two s", two=2)
                        for k in (2 * c, 2 * c + 1):
                            nc.tensor.matmul(
                                sv[:, k % 2, off:off + w],
                                kv[:, :, k * 128:(k + 1) * 128],
                                xv[:, :, start_col:start_col + w],
                                start=True, stop=True, perf_mode=DR)
                        ep = epool.tile([128, 2 * SC], FP8, tag="ep",
                                        name=f"ep{h}{j}{c}")
                        ev = ep[:].rearrange("p (two s) -> p two s", two=2)
                        nc.scalar.activation(
                            out=ev[:, :, off:off + w],
                            in_=sv[:, :, off:off + w],
                            func=AF.Exp, scale=1.0 / 1024.0, bias=ln16_t[:])
                        if c >= 2 * j:  # diagonal pair: causal masks
                            nc.vector.tensor_mul(
                                out=ep[:, off:off + 128],
                                in0=ep[:, off:off + 128], in1=cmask[:])
                            nc.vector.tensor_mul(
                                out=ep[:, SC + off:SC + off + 256],
                                in0=ep[:, SC + off:SC + off + 256],
                                in1=zcmask[:])
                        pend.append((ctx, c, ep, off, w))
                        if len(pend) > 2:
                            emit_zo(pend.pop(0))
                        it += 1
                        want = (len(nxt) * it + 5) // 6 if nxt else 0
                        while ndone < want:
                            nxt[ndone]()
                            ndone += 1
                while ndone < len(nxt):
                    nxt[ndone]()
                    ndone += 1
            for item in pend:
                emit_zo(item)
            warm_pe(8)

            # ================= wo + LN1 =================
            # Prefetch the sqrt_and_others ACT table while the PE runs wo:
            # a dummy sqrt hoists the single ACT_TABLE_LOAD of the LN phase
            # out of the LN chains (all later ACT funcs live in that set).
            r1b = [acts.tile([128, S], BF16, tag=f"r1b{d0}", name=f"r1b{d0}")
                   for d0 in range(ND)]
            x1b = [acts.tile([128, S], BF16, tag=f"x1b{d0}", name=f"x1b{d0}")
                   for d0 in range(ND)]

            def ln_stats(lo, w, src_b):
                """mean / mean-square stats matmuls for cols [lo, lo+w).
                Returns (mup, m2p, w) PSUM tiles (psO pool)."""
                cols = slice(lo, lo + w)
                sq = []
                for d0 in range(ND):
                    sqt = work.tile([128, SC], BF16, tag=f"sq{d0}",
                                    name=f"sq{lo}{d0}")
                    nc.gpsimd.tensor_mul(
                        out=sqt[:, 0:w], in0=src_b[d0][:, cols],
                        in1=src_b[d0][:, cols])
                    sq.append(sqt)
                mup = psO.tile([128, SC], F32, tag="o", name=f"mu{lo}")
                for d0 in range(ND):
                    nc.tensor.matmul(mup[:, 0:w], invd[:],
                                     src_b[d0][:, cols],
                                     start=(d0 == 0), stop=(d0 == ND - 1))
                m2p = psO.tile([128, SC], F32, tag="o", name=f"m2{lo}")
                for d0 in range(ND):
                    nc.tensor.matmul(m2p[:, 0:w], invd[:], sq[d0][:, 0:w],
                                     start=(d0 == 0), stop=(d0 == ND - 1))
                return mup, m2p, w

            def ln_chain2(stats_list, tag):
                """Chains for the given chunks, interleaved so their
                pipelines overlap: musq -> var -> sqrt -> recip.
                (Sqrt + DVE reciprocal keeps ACT inside sqrt_and_others.)"""
                js = list(range(len(stats_list)))
                ws = [st[2] for st in stats_list]
                musq, var, sd, rstd = {}, {}, {}, {}
                for j in js:
                    w = ws[j]
                    musq[j] = work.tile([128, SC], F32, tag=f"musq{j}",
                                        name="musq")
                    nc.scalar.activation(out=musq[j][:, 0:w],
                                         in_=stats_list[j][0][:, 0:w],
                                         func=AF.Square)
                for j in js:
                    w = ws[j]
                    var[j] = work.tile([128, SC], F32, tag=f"var{j}",
                                       name="var")
                    nc.vector.tensor_sub(out=var[j][:, 0:w],
                                         in0=stats_list[j][1][:, 0:w],
                                         in1=musq[j][:, 0:w])
                for j in js:
                    w = ws[j]
                    sd[j] = work.tile([128, SC], F32, tag=f"sd{j}",
                                      name="sd")
                    nc.scalar.activation(out=sd[j][:, 0:w],
                                         in_=var[j][:, 0:w],
                                         func=AF.Sqrt, bias=eps_t[:])
                for j in js:
                    w = ws[j]
                    rstd[j] = acts.tile([128, SC], F32, tag=f"rstd{tag}{j}",
                                        name=f"rstd{tag}{j}")
                    nc.vector.reciprocal_approx_fast(out=rstd[j][:, 0:w],
                                                     in_=sd[j][:, 0:w])
                return [rstd[j] for j in js]

            def cen_mm(lo, w, d0, src_b):
                """(src - mean(src)) for cols [lo, lo+w) half d0 via the
                centering matmul (I - J/D) on the otherwise-idle PE."""
                cols = slice(lo, lo + w)
                cp = psO.tile([128, SC], F32, tag="o", name=f"cen{lo}{d0}")
                nc.tensor.matmul(cp[:, 0:w], cent[:, 0:128],
                                 src_b[d0][:, cols], start=True, stop=False)
                nc.tensor.matmul(cp[:, 0:w], cent[:, 128:256],
                                 src_b[1 - d0][:, cols], start=False,
                                 stop=True)
                return cp

            def cen_apply(w, d0, cp, rstd, g, b, dst_ap):
                """dst = centered*g*rstd + b: one DVE stt + one ACT copy."""
                t = work.tile([128, SC], F32, tag=f"lt{d0}", name="lt")
                nc.vector.scalar_tensor_tensor(
                    out=t[:, 0:w], in0=cp[:, 0:w], scalar=g,
                    in1=rstd[:, 0:w], op0=ALU.mult, op1=ALU.mult)
                nc.scalar.activation(out=dst_ap, in_=t[:, 0:w],
                                     func=AF.Identity, bias=b)

            st1 = []
            for j in range(NJ):
                pair = psP.tile([128, 1024], F32, tag="pair", name=f"wo{j}")
                for d0 in range(ND):
                    for h in range(H):
                        wov = wo[h].rearrange("p (two d) -> p two d", two=2)
                        onv = ontp[h][:].rearrange("p (two s) -> p two s",
                                                   two=2)
                        nc.tensor.matmul(
                            pair[:, d0 * SC:d0 * SC + SC],
                            wov[:, :, d0 * 128:(d0 + 1) * 128],
                            onv[:, :, j * SC:(j + 1) * SC],
                            start=(h == 0), stop=(h == H - 1), perf_mode=DR)
                cols = slice(j * SC, (j + 1) * SC)
                for d0 in range(ND):
                    nc.vector.scalar_tensor_tensor(
                        out=r1b[d0][:, cols], in0=pair[:, d0 * SC:d0 * SC + SC],
                        scalar=1.0 / 4096.0, in1=xbt[d0][:, cols],
                        op0=ALU.mult, op1=ALU.add)
                st1.append(ln_stats(j * SC, SC, r1b))
                if j == 0:
                    # sqrt-table prefetch, anchored after attention: reads
                    # r1b so the scheduler keeps it out of the exp stream
                    sqscr = consts.tile([128, 1], F32, tag="sqscr",
                                        name="sqscr")
                    nc.scalar.activation(out=sqscr[:], in_=r1b[0][:, 0:1],
                                         func=AF.Sqrt, scale=0.0,
                                         bias=eps_t[:])
            warm_pe(22)
            ch1 = ln_chain2(st1, "a")
            cen1 = [[cen_mm(j * SC, SC, d0, r1b) for d0 in range(ND)]
                    for j in range(NJ)]
            for j in range(NJ):
                cols = slice(j * SC, (j + 1) * SC)
                for d0 in range(ND):
                    cen_apply(SC, d0, cen1[j][d0], ch1[j], ln1_g[d0],
                              ln1_b[d0], x1b[d0][:, cols])

            # ================= FFN =================
            hp = acts.tile([128, NF * S], FP8, tag="hp", name="hp")
            warm_pe(10)
            for j in range(NJ):
                cols = slice(j * SC, (j + 1) * SC)
                for c in range(NF // 2):
                    pair = psP.tile([128, 1024], F32, tag="pair",
                                    name=f"h{j}{c}")
                    for t in range(2):
                        f0 = 2 * c + t
                        for d0 in range(ND):
                            nc.tensor.matmul(
                                pair[:, t * SC:t * SC + SC],
                                f1[d0][:, f0 * 128:(f0 + 1) * 128],
                                x1b[d0][:, cols],
                                start=(d0 == 0), stop=(d0 == ND - 1))
                    # relu evac: one per f0 (separate per-partition biases)
                    for t in range(2):
                        f0 = 2 * c + t
                        dst = hp[:, f0 * S + j * SC:f0 * S + (j + 1) * SC]
                        if c % 2 == 0:
                            nc.scalar.activation(
                                out=dst, in_=pair[:, t * SC:t * SC + SC],
                                func=AF.Relu, bias=f1b[f0])
                        else:
                            nc.vector.tensor_scalar(
                                out=dst, in0=pair[:, t * SC:t * SC + SC],
                                scalar1=f1b[f0], scalar2=0.0,
                                op0=ALU.add, op1=ALU.max)

            warm_pe(6)
            r2b = [acts.tile([128, S], BF16, tag=f"r2b{d0}", name=f"r2b{d0}")
                   for d0 in range(ND)]
            hv = hp[:].rearrange("p (f s) -> p f s", f=NF)
            ch2 = [None, None]
            cen2 = [None, None]
            for j in range(NJ):
                cols = slice(j * SC, (j + 1) * SC)
                pair = psP.tile([128, 1024], F32, tag="pair", name=f"ff{j}")
                for d0 in range(ND):
                    for c in range(NF // 2):
                        f2v = f2[c].rearrange("p (two d) -> p two d", two=2)
                        nc.tensor.matmul(
                            pair[:, d0 * SC:d0 * SC + SC],
                            f2v[:, :, d0 * 128:(d0 + 1) * 128],
                            hv[:, 2 * c:2 * c + 2, cols],
                            start=(c == 0), stop=(c == NF // 2 - 1),
                            perf_mode=DR)
                for d0 in range(ND):
                    nc.vector.scalar_tensor_tensor(
                        out=r2b[d0][:, cols], in0=pair[:, d0 * SC:d0 * SC + SC],
                        scalar=1.0 / 4096.0, in1=x1b[d0][:, cols],
                        op0=ALU.mult, op1=ALU.add)
                # stats + chain + centering per chunk immediately: chunk
                # 0's chain hides under chunk 1's FFN2 matmuls; the last
                # chunk is split into two 256-col halves so its exposed
                # serial chain is half as long.
                if j == 0:
                    ch2[0] = ln_chain2([ln_stats(0, SC, r2b)], "b0")
                    cen2[0] = [cen_mm(0, SC, d0, r2b) for d0 in range(ND)]
                    warm_pe(4)
                else:
                    sts = [ln_stats(SC, 256, r2b), ln_stats(SC + 256, 256,
                                                            r2b)]
                    ch2[1] = ln_chain2(sts, "b1")
                    cen2[1] = [[cen_mm(SC + 256 * half, 256, d0, r2b)
                                for d0 in range(ND)] for half in range(2)]
            # output applies + DMA
            for d0 in range(ND):
                ot = outp.tile([128, SC], F32, tag=f"out0{d0}",
                               name=f"out0{d0}")
                cen_apply(SC, d0, cen2[0][d0], ch2[0][0], ln2_g[d0],
                          ln2_b[d0], ot[:])
                nc.sync.dma_start(out=out_d[d0][:, 0:SC], in_=ot[:])
            for half in range(2):
                for d0 in range(ND):
                    ot = outp.tile([128, 256], F32, tag=f"out1{half}{d0}",
                                   name=f"out1{half}{d0}")
                    cen_apply(256, d0, cen2[1][half][d0], ch2[1][half],
                              ln2_g[d0], ln2_b[d0], ot[:])
                    lo = SC + 256 * half
                    nc.sync.dma_start(out=out_d[d0][:, lo:lo + 256],
                                      in_=ot[:])

    nc.compile()
    return nc


def _np_reference(x, attention_mask, wq, wk, wv, wo_w, wo_b, ln1_g, ln1_b,
                  ff1_w, ff1_b, ff2_w, ff2_b, ln2_g, ln2_b):
    """Numpy fallback (only used if attention_mask has zeros)."""
    def ln(t, g, b):
        mu = t.mean(-1, keepdims=True)
        var = t.var(-1, keepdims=True)
        return (t - mu) / np.sqrt(var + LN_EPS) * g + b
    Bn, Sn, Dn = x.shape
    q = np.einsum('bsd,hed->bhse', x, wq)
    k = np.einsum('bsd,hed->bhse', x, wk)
    v = np.einsum('bsd,hed->bhse', x, wv)
    sc = np.einsum('bhse,bhte->bhst', q, k) / np.sqrt(np.float32(Dn))
    idx = np.arange(Sn)
    causal = idx[None, :] > idx[:, None]
    m = attention_mask.astype(bool)
    valid = m[:, None, :] & m[:, :, None]
    cond = causal[None] | ~valid
    sc = np.where(cond[:, None], -np.inf, sc)
    sc = sc - np.nanmax(np.where(np.isinf(sc), np.nan, sc), axis=-1,
                        keepdims=True)
    e = np.exp(sc)
    e = np.where(np.isnan(e), 0.0, e)
    att = e / np.maximum(e.sum(-1, keepdims=True), 1e-30)
    ho = np.einsum('bhst,bhte->bhse', att, v)
    cat = np.transpose(ho, (0, 2, 1, 3)).reshape(Bn, Sn, -1)
    mh = cat @ wo_w.T + wo_b
    x1 = ln(x + mh, ln1_g, ln1_b)
    hh = np.maximum(x1 @ ff1_w.T + ff1_b, 0.0)
    ff = hh @ ff2_w.T + ff2_b
    return ln(x1 + ff, ln2_g, ln2_b).astype(np.float32)


def _fp8(a):
    return np.clip(a, -FP8MAX, FP8MAX).astype(ml_dtypes.float8_e4m3)


def _prep_inputs(inputs):
    bf = ml_dtypes.bfloat16
    f32 = np.float32
    x = np.asarray(inputs["x"], f32)            # [B, S, D]
    wq = np.asarray(inputs["wq"], f32)          # [H, E, D]
    wk = np.asarray(inputs["wk"], f32)
    wvw = np.asarray(inputs["wv"], f32)
    wo_w = np.asarray(inputs["wo_w"], f32)      # [D, HE]
    wo_b = np.asarray(inputs["wo_b"], f32)
    ff1 = np.asarray(inputs["ff1_w"], f32)      # [FF, D]
    ff1_b = np.asarray(inputs["ff1_b"], f32)
    ff2 = np.asarray(inputs["ff2_w"], f32)      # [D, FF]
    ff2_b = np.asarray(inputs["ff2_b"], f32)
    ln1_g = np.asarray(inputs["ln1_g"], f32)
    ln1_b = np.asarray(inputs["ln1_b"], f32)
    ln2_g = np.asarray(inputs["ln2_g"], f32)
    ln2_b = np.asarray(inputs["ln2_b"], f32)

    def pack2(a):  # [X(=2*128), C] -> [128, 2, C] -> [128, 2*C]
        X, C = a.shape
        return np.ascontiguousarray(
            a.reshape(2, 128, C).transpose(1, 0, 2).reshape(128, 2 * C))

    # wpk: mq0|wv0|...|mq3|wv3|wo0..wo3 packed [128, 12*512] fp8
    wpk = np.zeros((128, 12 * 512), ml_dtypes.float8_e4m3)
    for h in range(H):
        # mq: stationary for kt = M x, M_h = wq_h^T wk_h (contraction d')
        M = (wq[h].astype(np.float64).T @ wk[h].astype(np.float64)).astype(f32)
        wpk[:, (2 * h) * 512:(2 * h) * 512 + 512] = _fp8(
            pack2(1024.0 * M.T))                # [d', d] layout
        # wv moving: [d-pairs, e], x64
        wpk[:, (2 * h + 1) * 512:(2 * h + 1) * 512 + 512] = _fp8(
            pack2(64.0 * wvw[h].T))
        # wo stationary: per head, [e-pairs(of head), d], x64
        blk = wo_w[:, h * E:(h + 1) * E].T      # [E, D]
        wpk[:, (8 + h) * 512:(8 + h) * 512 + 512] = _fp8(pack2(64.0 * blk))
    # f1 stationary bf16 x64: [128, 2*FF]
    f1p = np.ascontiguousarray(
        (64.0 * ff1).T.reshape(2, 128, FF).reshape(
            2, 128, FF).transpose(1, 0, 2).reshape(128, 2 * FF)).astype(bf)
    # f2 stationary DR pairs x64: [128, 4*2*D]
    f2t = (64.0 * ff2).T.reshape(4, 2, 128, D).transpose(2, 0, 1, 3)
    f2p = _fp8(np.ascontiguousarray(f2t.reshape(128, 8 * D)))
    # centering matrix [A | Bm]: A = I - J/D, Bm = -J/D (bf16-exact values)
    cen = np.zeros((128, 256), f32)
    cen[:, 0:128] = np.eye(128, dtype=f32) - 1.0 / D
    cen[:, 128:256] = -1.0 / D
    cen = cen.astype(bf)
    # packed consts [128, 16]
    cs = np.zeros((128, 16), f32)
    ln1_b_eff = ln1_b + ff2_b
    f1b_eff = 64.0 * (ff1_b - ff1 @ ff2_b)
    cs[:, 0:2] = ln1_g.reshape(2, 128).T
    cs[:, 2:4] = ln1_b_eff.reshape(2, 128).T
    cs[:, 4:6] = ln2_g.reshape(2, 128).T
    cs[:, 6:8] = ln2_b.reshape(2, 128).T
    cs[:, 8:16] = f1b_eff.reshape(8, 128).T

    shared = dict(wpk=wpk, f1=f1p, f2=f2p, cs=cs, cen=cen)
    in_maps = []
    for b in range(B):
        xT = np.ascontiguousarray(x[b].T)       # [D, S]
        m = dict(shared)
        m["xp"] = _fp8(pack2(xT))
        m["xb"] = np.ascontiguousarray(
            (xT + wo_b[:, None]).reshape(2, 128, S).transpose(1, 0, 2)
            .reshape(128, 2 * S)).astype(bf)
        in_maps.append(m)
    return in_maps


def run_sharded(inputs, trace=False, trace_kwargs=None):
    if "nc" not in _CACHE:
        _CACHE["nc"] = _build()
    nc = _CACHE["nc"]
    in_maps = _prep_inputs(inputs)
    res = run_bass_kernel_spmd(nc, in_maps, list(range(N_CORES)), trace=trace,
                               **(trace_kwargs or {}))
    outs = []
    for b in range(B):
        r = np.asarray(res.results[b]["out"], np.float32).reshape(D, S)
        outs.append(r.T)
    return np.stack(outs), res


def kernel(**inputs) -> np.ndarray:
    mask = np.asarray(inputs["attention_mask"])
    if not np.all(mask != 0):
        return _np_reference(**{k: np.asarray(v) for k, v in inputs.items()})
    out, _ = run_sharded(inputs, trace=False)
    return out

